# revision 10
# baseline (speedup 1.0000x reference)
"""Masked causal self-attention on 8 Trainium2 NeuronCores.

Sharding (Megatron-style): core c -> (batch b = c//4, head-group g = c%4).
Each core computes QKV projections for its 4 heads (512 of 2048 cols,
column-parallel), causal attention for those heads on its batch, and a
row-parallel slice of the output projection, producing a partial [S, D]
output. Host sums the 4 partials per batch and adds bp.

On-chip dataflow is fully transposed (feature-major) so no transposes are
ever needed:
  x^T (host-prepped)  --W as lhsT-->  Q^T, K^T [hd, S];  x^T as lhsT --> V [S, hd]
  S^T = (K^T tile).T @ Q^T            [Sk part, Sq free]
  attn^T = exp(S^T * scale) * mask    (no max subtraction: |scores| < ~1)
  rowsum = ones.T @ attn^T            (PE, M=1)
  O^T += (V tile).T @ attn^T          [hd part, Sq free]
  out_partial = (O^T tile).T @ Wp     [S part, D free]
Matmuls in bf16 (4x fp32 TensorE throughput), fp32 PSUM accumulation,
fully masked causal blocks skipped.
"""

import os
import sys

import numpy as np

try:
    import concourse.bass as bass
except ImportError:
    sys.path.insert(0, "/opt/trn_rl_repo")
    import concourse.bass as bass

import ml_dtypes
import concourse.mybir as mybir
import concourse.tile as tile
from concourse.bass_utils import run_bass_kernel_spmd

BF16 = mybir.dt.bfloat16
F32 = mybir.dt.float32
AF = mybir.ActivationFunctionType

B, S, D, H, HD = 2, 2048, 2048, 16, 128
NH = 4                # heads per core
HG = NH * HD          # 512: head-group width per core
NKT = D // 128        # 16 contraction k-tiles over D
NST = S // 128        # 16 s-tiles of 128
NQC = S // 512        # 4 q-chunks of 512
SCALE = 1.0 / float(np.sqrt(D))

LAST_EXEC_NS = None


def split_excess_waits(nc, maxw=1):
    """Walrus in this toolchain rejects >1 sync wait on CTRL-class
    instructions (Tile's tail drain can carry many). Hoist excess waits
    onto preceding single-wait EventSemaphore instructions."""
    for f in nc.m.functions:
        for bb in f.blocks:
            out, changed, k = [], False, 0
            for inst in bb.instructions:
                si = inst.sync_info
                if si is not None and len(si.on_wait) > maxw:
                    waits = list(si.on_wait)
                    while len(waits) > maxw:
                        chunk, waits = waits[:maxw], waits[maxw:]
                        out.append(mybir.InstEventSemaphore(
                            name=f"{inst.name}-waitsplit{k}", engine=inst.engine,
                            sync_info=mybir.SyncInfo(on_wait=chunk, on_update=[])))
                        k += 1
                        changed = True
                    si.on_wait = waits
                out.append(inst)
            if changed:
                bb.instructions = out


def build(trace_friendly=False):
    nc = bass.Bass()

    xT = nc.declare_dram_parameter("xT", [D, S], BF16, isOutput=False)
    wq = nc.declare_dram_parameter("wq", [D, HG], BF16, isOutput=False)
    wk = nc.declare_dram_parameter("wk", [D, HG], BF16, isOutput=False)
    wv = nc.declare_dram_parameter("wv", [D, HG], BF16, isOutput=False)
    wp = nc.declare_dram_parameter("wp", [HG, D], BF16, isOutput=False)
    bqk = nc.declare_dram_parameter("bqk", [128, 2 * NH], F32, isOutput=False)
    bv = nc.declare_dram_parameter("bv", [128, HG], F32, isOutput=False)
    masks = nc.declare_dram_parameter("masks", [128, 4 * 512], BF16, isOutput=False)
    ones_col = nc.declare_dram_parameter("ones_col", [128, 1], BF16, isOutput=False)
    ones_row = nc.declare_dram_parameter("ones_row", [1, 128], F32, isOutput=False)
    out = nc.declare_dram_parameter("out", [S, D], F32, isOutput=True)

    with tile.TileContext(nc) as tc:
        with tc.tile_pool(name="const", bufs=1) as cpool, \
             tc.tile_pool(name="qkv", bufs=1) as qkv_pool:
            bqk_sb = cpool.tile([128, 2 * NH], F32, tag="bqk")
            nc.sync.dma_start(bqk_sb[:], bqk[:])
            bv_sb = cpool.tile([128, HG], F32, tag="bv")
            nc.sync.dma_start(bv_sb[:], bv[:])
            mask_sb = cpool.tile([128, 4 * 512], BF16, tag="masks")
            nc.sync.dma_start(mask_sb[:], masks[:])
            onec_sb = cpool.tile([128, 1], BF16, tag="onec")
            nc.sync.dma_start(onec_sb[:], ones_col[:])
            oner_sb = cpool.tile([1, 128], F32, tag="oner")
            nc.sync.dma_start(oner_sb[:], ones_row[:])

            # Per-head feature-major Q^T/K^T/O^T: head h lives in cols
            # [h*S, (h+1)*S). V is token-major: s-tile st in cols
            # [st*HG, (st+1)*HG).
            QT = qkv_pool.tile([128, NH * S], BF16, tag="QT")
            KT = qkv_pool.tile([128, NH * S], BF16, tag="KT")
            V = qkv_pool.tile([128, NST * HG], BF16, tag="V")
            OT = qkv_pool.tile([128, NH * S], BF16, tag="OT")

            # ---------------- Phase 1: QKV projections ----------------
            with tc.tile_pool(name="xw", bufs=1) as xw_pool, \
                 tc.tile_pool(name="ps1", bufs=4, space="PSUM") as ps1:
                xt_t = []
                for kt in range(NKT):
                    t = xw_pool.tile([128, S], BF16, tag=f"xt{kt}")
                    nc.sync.dma_start(t[:], xT[kt * 128:(kt + 1) * 128, :])
                    xt_t.append(t)
                w_t = {}
                for nm, dram in (("q", wq), ("k", wk), ("v", wv)):
                    for kt in range(NKT):
                        t = xw_pool.tile([128, HG], BF16, tag=f"w{nm}{kt}")
                        nc.sync.dma_start(t[:], dram[kt * 128:(kt + 1) * 128, :])
                        w_t[nm, kt] = t

                # Q^T and K^T: [hd' m-tile 128][Sq chunk 512] = W.T @ x^T
                for nm, dstT, bcol in (("q", QT, 0), ("k", KT, NH)):
                    for m in range(NH):
                        for nq in range(NQC):
                            acc = ps1.tile([128, 512], F32, tag="ps1")
                            for kt in range(NKT):
                                nc.tensor.matmul(
                                    acc[:],
                                    w_t[nm, kt][:, m * 128:(m + 1) * 128],
                                    xt_t[kt][:, nq * 512:(nq + 1) * 512],
                                    start=(kt == 0), stop=(kt == NKT - 1),
                                )
                            nc.scalar.activation(
                                dstT[:, m * S + nq * 512: m * S + nq * 512 + 512],
                                acc[:], AF.Identity,
                                bias=bqk_sb[:, bcol + m: bcol + m + 1],
                            )
                # V (token-major): x^T tile as lhsT
                for st in range(NST):
                    acc = ps1.tile([128, 512], F32, tag="ps1")
                    for kt in range(NKT):
                        nc.tensor.matmul(
                            acc[:],
                            xt_t[kt][:, st * 128:(st + 1) * 128],
                            w_t["v", kt][:],
                            start=(kt == 0), stop=(kt == NKT - 1),
                        )
                    nc.vector.tensor_add(
                        V[:, st * HG:(st + 1) * HG], acc[:], bv_sb[:])

            # ---------------- Phase 2: causal attention ----------------
            with tc.tile_pool(name="wp_pool", bufs=1) as wp_pool:
                wp_t = []
                for h in range(NH):
                    t = wp_pool.tile([128, D], BF16, tag=f"wp{h}")
                    nc.sync.dma_start(t[:], wp[h * 128:(h + 1) * 128, :])
                    wp_t.append(t)

                with tc.tile_pool(name="attn", bufs=4) as attn_pool, \
                     tc.tile_pool(name="fin", bufs=2) as fin_pool, \
                     tc.tile_pool(name="ps_s", bufs=3, space="PSUM") as ps_s, \
                     tc.tile_pool(name="ps_o", bufs=2, space="PSUM") as ps_o, \
                     tc.tile_pool(name="ps_r", bufs=2, space="PSUM") as ps_r, \
                     tc.tile_pool(name="ps_b", bufs=1, space="PSUM") as ps_b:
                    self_attn(nc, tc, attn_pool, fin_pool, ps_s, ps_o, ps_r,
                              ps_b, QT, KT, V, OT, mask_sb, onec_sb, oner_sb)
                # ---------------- Phase 3: output projection ----------------
                with tc.tile_pool(name="outst", bufs=4) as outst, \
                     tc.tile_pool(name="ps3", bufs=4, space="PSUM") as ps3:
                    for ms in range(NST):
                        for nc2 in range(NQC):
                            acc = ps3.tile([128, 512], F32, tag="ps3")
                            for h in range(NH):
                                nc.tensor.matmul(
                                    acc[:],
                                    OT[:, h * S + ms * 128: h * S + ms * 128 + 128],
                                    wp_t[h][:, nc2 * 512:(nc2 + 1) * 512],
                                    start=(h == 0), stop=(h == NH - 1),
                                )
                            ot = outst.tile([128, 512], F32, tag="outst")
                            nc.scalar.copy(ot[:], acc[:])
                            nc.sync.dma_start(
                                out[ms * 128:(ms + 1) * 128,
                                    nc2 * 512:(nc2 + 1) * 512], ot[:])
    split_excess_waits(nc)
    return nc


def self_attn(nc, tc, attn_pool, fin_pool, ps_s, ps_o, ps_r, ps_b,
              QT, KT, V, OT, mask_sb, onec_sb, oner_sb):
    for h in range(NH):
        hS = h * S
        for qc in range(NQC):
            q0 = qc * 512
            kt_lim = 4 * (qc + 1)
            acc_o = ps_o.tile([128, 512], F32, tag="ps_o")
            acc_r = ps_r.tile([1, 512], F32, tag="ps_r")
            for kt in range(kt_lim):
                ps = ps_s.tile([128, 512], F32, tag="ps_s")
                nc.tensor.matmul(
                    ps[:],
                    KT[:, hS + kt * 128: hS + kt * 128 + 128],
                    QT[:, hS + q0: hS + q0 + 512],
                    start=True, stop=True,
                )
                at = attn_pool.tile([128, 512], BF16, tag="at")
                nc.scalar.activation(at[:], ps[:], AF.Exp, scale=SCALE)
                r = kt - 4 * qc
                if r >= 0:  # staircase block: apply causal mask
                    nc.vector.tensor_mul(
                        at[:], at[:], mask_sb[:, r * 512:(r + 1) * 512])
                first, last = kt == 0, kt == kt_lim - 1
                nc.tensor.matmul(acc_r[:], onec_sb[:], at[:],
                                 start=first, stop=last)
                nc.tensor.matmul(
                    acc_o[:],
                    V[:, kt * HG + h * 128: kt * HG + h * 128 + 128],
                    at[:], start=first, stop=last)
            # normalize: O^T[:, i] /= rowsum[i]
            rs = fin_pool.tile([1, 512], F32, tag="rs")
            nc.vector.reciprocal(rs[:], acc_r[:])
            bc = ps_b.tile([128, 512], F32, tag="ps_b")
            nc.tensor.matmul(bc[:], oner_sb[:], rs[:], start=True, stop=True)
            rcp = fin_pool.tile([128, 512], F32, tag="rcp")
            nc.scalar.copy(rcp[:], bc[:])
            nc.vector.tensor_mul(
                OT[:, hS + q0: hS + q0 + 512], acc_o[:], rcp[:])


_NC_CACHE = None


def _get_nc():
    global _NC_CACHE
    if _NC_CACHE is None:
        _NC_CACHE = build()
    return _NC_CACHE


def _prep_in_maps(x, Wq, bq, Wk, bk, Wv, bv, Wp, bp):
    x = np.asarray(x, dtype=np.float32)
    bf = ml_dtypes.bfloat16
    # causal staircase masks: mask_r[j, i] = 1 if i >= j + r*128
    jj = np.arange(128)[:, None]
    ii = np.arange(512)[None, :]
    masks = np.concatenate(
        [(ii >= jj + r * 128).astype(np.float32) for r in range(4)], axis=1
    ).astype(bf)
    ones_col = np.ones((128, 1), dtype=bf)
    ones_row = np.ones((1, 128), dtype=np.float32)

    xTb = [np.ascontiguousarray(x[b].T).astype(bf) for b in range(B)]
    in_maps = []
    for c in range(8):
        b, g = divmod(c, 4)
        sl = slice(g * HG, (g + 1) * HG)
        bqk = np.concatenate(
            [np.asarray(bq)[sl].reshape(NH, 128).T,
             np.asarray(bk)[sl].reshape(NH, 128).T], axis=1
        ).astype(np.float32)
        bv_rep = np.broadcast_to(
            np.asarray(bv)[sl].astype(np.float32), (128, HG)).copy()
        in_maps.append({
            "xT": xTb[b],
            "wq": np.ascontiguousarray(np.asarray(Wq)[:, sl]).astype(bf),
            "wk": np.ascontiguousarray(np.asarray(Wk)[:, sl]).astype(bf),
            "wv": np.ascontiguousarray(np.asarray(Wv)[:, sl]).astype(bf),
            "wp": np.ascontiguousarray(np.asarray(Wp)[sl, :]).astype(bf),
            "bqk": bqk,
            "bv": bv_rep,
            "masks": masks,
            "ones_col": ones_col,
            "ones_row": ones_row,
        })
    return in_maps


def kernel(x, Wq, bq, Wk, bk, Wv, bv, Wp, bp):
    global LAST_EXEC_NS
    nc = _get_nc()
    in_maps = _prep_in_maps(x, Wq, bq, Wk, bk, Wv, bv, Wp, bp)
    res = run_bass_kernel_spmd(nc, in_maps, core_ids=list(range(8)))
    LAST_EXEC_NS = res.exec_time_ns
    out = np.empty((B, S, D), dtype=np.float32)
    for b in range(B):
        acc = res.results[4 * b]["out"].astype(np.float32)
        for g in range(1, 4):
            acc = acc + res.results[4 * b + g]["out"]
        out[b] = acc
    out += np.asarray(bp, dtype=np.float32)[None, None, :]
    return out


def _make_runner(nc, in_maps, donate):
    """Replicate bass2jax.run_bass_via_pjrt's shard_map jit, returning a
    zero-arg callable over device-resident inputs (for repeat timing)."""
    import jax
    from jax.sharding import Mesh, PartitionSpec, NamedSharding
    from jax.experimental.shard_map import shard_map
    from concourse import bass2jax, mybir as _mybir
    from concourse.bass2jax import _bass_exec_p, install_neuronx_cc_hook

    install_neuronx_cc_hook()
    n_cores = len(in_maps)
    partition_name = (nc.partition_id_tensor.name
                      if nc.partition_id_tensor else None)
    in_names, out_names, out_avals, zero_outs = [], [], [], []
    for alloc in nc.m.functions[0].allocations:
        if not isinstance(alloc, _mybir.MemoryLocationSet):
            continue
        name = alloc.memorylocations[0].name
        if alloc.kind == "ExternalInput":
            if name != partition_name:
                in_names.append(name)
        elif alloc.kind == "ExternalOutput":
            out_names.append(name)
            shape = tuple(alloc.tensor_shape)
            dtype = _mybir.dt.np(alloc.dtype)
            out_avals.append(jax.core.ShapedArray(shape, dtype))
            zero_outs.append(np.zeros(shape, dtype))
    n_params = len(in_names)
    n_outs = len(out_avals)
    in_names = in_names + out_names
    if partition_name is not None:
        in_names.append(partition_name)

    def _body(*args):
        operands = list(args)
        if partition_name is not None:
            operands.append(bass2jax.partition_id_tensor())
        outs = _bass_exec_p.bind(
            *operands, out_avals=tuple(out_avals), in_names=tuple(in_names),
            out_names=tuple(out_names), lowering_input_output_aliases=(),
            sim_require_finite=True, sim_require_nnan=True, nc=nc)
        return tuple(outs)

    devices = jax.devices()[:n_cores]
    mesh = Mesh(np.asarray(devices), ("core",))
    in_specs = (PartitionSpec("core"),) * (n_params + n_outs)
    out_specs = (PartitionSpec("core"),) * len(out_names)
    fn = jax.jit(
        shard_map(_body, mesh=mesh, in_specs=in_specs, out_specs=out_specs,
                  check_rep=False),
        keep_unused=True)
    sh = NamedSharding(mesh, PartitionSpec("core"))
    concat_in = [
        jax.device_put(
            np.concatenate([np.asarray(in_maps[c][in_names[i]])
                            for c in range(n_cores)], axis=0), sh)
        for i in range(n_params)
    ]
    concat_zeros = [
        jax.device_put(np.zeros((n_cores * z.shape[0], *z.shape[1:]), z.dtype), sh)
        for z in zero_outs
    ]
    args = concat_in + concat_zeros

    def run():
        return fn(*args)

    return run


def benchmark_floor(iters=20):
    """RPC/dispatch floor: time a trivial 8-core kernel the same way."""
    import time
    import jax
    nc = bass.Bass()
    xi = nc.declare_dram_parameter("x", [128, 128], F32, isOutput=False)
    oo = nc.declare_dram_parameter("o", [128, 128], F32, isOutput=True)
    with tile.TileContext(nc) as tc:
        with tc.tile_pool(name="sb", bufs=1) as sb:
            t = sb.tile([128, 128], F32, tag="t")
            nc.sync.dma_start(t[:], xi[:])
            nc.sync.dma_start(oo[:], t[:])
    split_excess_waits(nc)
    in_maps = [{"x": np.zeros((128, 128), np.float32)} for _ in range(8)]
    run = _make_runner(nc, in_maps, donate=False)
    jax.block_until_ready(run())
    times = []
    for _ in range(iters):
        t0 = time.perf_counter()
        jax.block_until_ready(run())
        times.append(time.perf_counter() - t0)
    times.sort()
    med = times[len(times) // 2]
    print(f"floor: min {min(times)*1e6:.0f} us, median {med*1e6:.0f} us")
    return med * 1e9


def benchmark(inputs, iters=20):
    """Median wall time per execution with device-resident inputs, in ns."""
    import time
    import jax
    nc = _get_nc()
    in_maps = _prep_in_maps(**inputs)
    run = _make_runner(nc, in_maps, donate=False)
    out = run()
    jax.block_until_ready(out)  # compile + warm
    times = []
    for _ in range(iters):
        t0 = time.perf_counter()
        jax.block_until_ready(run())
        times.append(time.perf_counter() - t0)
    times.sort()
    med = times[len(times) // 2]
    print(f"benchmark: min {min(times)*1e6:.0f} us, median {med*1e6:.0f} us, "
          f"max {max(times)*1e6:.0f} us over {iters} iters")
    return med * 1e9


# revision 13
# speedup vs baseline: 156.1094x; 156.1094x over previous
"""Masked causal self-attention on 8 Trainium2 NeuronCores.

Sharding (Megatron-style): core c -> (batch b = c//4, head-group g = c%4).
Each core computes QKV projections for its 4 heads (512 of 2048 cols,
column-parallel), causal attention for those heads on its batch, and a
row-parallel slice of the output projection, producing a partial [S, D]
output. Host sums the 4 partials per batch and adds bp.

On-chip dataflow is fully transposed (feature-major) so no transposes are
ever needed:
  x^T (host-prepped)  --W as lhsT-->  Q^T, K^T [hd, S];  x^T as lhsT --> V [S, hd]
  S^T = (K^T tile).T @ Q^T            [Sk part, Sq free]
  attn^T = exp(S^T * scale) * mask    (no max subtraction: |scores| < ~1)
  rowsum = ones.T @ attn^T            (PE, M=1)
  O^T += (V tile).T @ attn^T          [hd part, Sq free]
  out_partial = (O^T tile).T @ Wp     [S part, D free]
Matmuls in bf16 (4x fp32 TensorE throughput), fp32 PSUM accumulation,
fully masked causal blocks skipped.
"""

import os
import sys

import numpy as np

try:
    import concourse.bass as bass
except ImportError:
    sys.path.insert(0, "/opt/trn_rl_repo")
    import concourse.bass as bass

import ml_dtypes
import concourse.mybir as mybir
import concourse.tile as tile
from concourse.bass_utils import run_bass_kernel_spmd

BF16 = mybir.dt.bfloat16
F32 = mybir.dt.float32
AF = mybir.ActivationFunctionType

B, S, D, H, HD = 2, 2048, 2048, 16, 128
NH = 4                # heads per core
HG = NH * HD          # 512: head-group width per core
NKT = D // 128        # 16 contraction k-tiles over D
NST = S // 128        # 16 s-tiles of 128
NQC = S // 512        # 4 q-chunks of 512
SCALE = 1.0 / float(np.sqrt(D))

LAST_EXEC_NS = None


def split_excess_waits(nc, maxw=1):
    """Walrus in this toolchain rejects >1 sync wait on CTRL-class
    instructions (Tile's tail drain can carry many). Hoist excess waits
    onto preceding single-wait EventSemaphore instructions."""
    for f in nc.m.functions:
        for bb in f.blocks:
            out, changed, k = [], False, 0
            for inst in bb.instructions:
                si = inst.sync_info
                if si is not None and len(si.on_wait) > maxw:
                    waits = list(si.on_wait)
                    while len(waits) > maxw:
                        chunk, waits = waits[:maxw], waits[maxw:]
                        out.append(mybir.InstEventSemaphore(
                            name=f"{inst.name}-waitsplit{k}", engine=inst.engine,
                            sync_info=mybir.SyncInfo(on_wait=chunk, on_update=[])))
                        k += 1
                        changed = True
                    si.on_wait = waits
                out.append(inst)
            if changed:
                bb.instructions = out


def qkv_proj(nc, tc, xT, wq, wk, wv, bqk_sb, bv_sb, QT, KT, V):
    """Phase 1: Q^T,K^T (feature-major) and V (token-major) projections."""
    with tc.tile_pool(name="xw", bufs=1) as xw_pool, \
         tc.tile_pool(name="ps1", bufs=4, space="PSUM") as ps1:
        xt_t = []
        for kt in range(NKT):
            t = xw_pool.tile([128, S], BF16, tag=f"xt{kt}")
            nc.sync.dma_start(t[:], xT[kt * 128:(kt + 1) * 128, :])
            xt_t.append(t)
        w_t = {}
        for nm, dram in (("q", wq), ("k", wk), ("v", wv)):
            for kt in range(NKT):
                t = xw_pool.tile([128, HG], BF16, tag=f"w{nm}{kt}")
                nc.sync.dma_start(t[:], dram[kt * 128:(kt + 1) * 128, :])
                w_t[nm, kt] = t

        # Q^T and K^T: [hd' m-tile 128][Sq chunk 512] = W.T @ x^T
        for nm, dstT, bcol in (("q", QT, 0), ("k", KT, NH)):
            for m in range(NH):
                for nq in range(NQC):
                    acc = ps1.tile([128, 512], F32, tag="ps1")
                    for kt in range(NKT):
                        nc.tensor.matmul(
                            acc[:],
                            w_t[nm, kt][:, m * 128:(m + 1) * 128],
                            xt_t[kt][:, nq * 512:(nq + 1) * 512],
                            start=(kt == 0), stop=(kt == NKT - 1),
                        )
                    nc.scalar.activation(
                        dstT[:, m * S + nq * 512: m * S + nq * 512 + 512],
                        acc[:], AF.Identity,
                        bias=bqk_sb[:, bcol + m: bcol + m + 1],
                    )
        # V (token-major): x^T tile as lhsT
        for st in range(NST):
            acc = ps1.tile([128, 512], F32, tag="ps1")
            for kt in range(NKT):
                nc.tensor.matmul(
                    acc[:],
                    xt_t[kt][:, st * 128:(st + 1) * 128],
                    w_t["v", kt][:],
                    start=(kt == 0), stop=(kt == NKT - 1),
                )
            nc.vector.tensor_add(V[:, st * HG:(st + 1) * HG], acc[:], bv_sb[:])


def self_attn(nc, tc, attn_pool, fin_pool, ps_s, ps_o, ps_r, ps_b,
              QT, KT, V, OT, mask_sb, onec_sb, oner_sb):
    """Phase 2: causal attention per head, transposed-scores flash style."""
    for h in range(NH):
        hS = h * S
        for qc in range(NQC):
            q0 = qc * 512
            kt_lim = 4 * (qc + 1)
            acc_o = ps_o.tile([128, 512], F32, tag="ps_o")
            acc_r = ps_r.tile([1, 512], F32, tag="ps_r")
            for kt in range(kt_lim):
                ps = ps_s.tile([128, 512], F32, tag="ps_s")
                nc.tensor.matmul(
                    ps[:],
                    KT[:, hS + kt * 128: hS + kt * 128 + 128],
                    QT[:, hS + q0: hS + q0 + 512],
                    start=True, stop=True,
                )
                at = attn_pool.tile([128, 512], BF16, tag="at")
                nc.scalar.activation(at[:], ps[:], AF.Exp, scale=SCALE)
                r = kt - 4 * qc
                if r >= 0:  # staircase block: apply causal mask
                    nc.vector.tensor_mul(
                        at[:], at[:], mask_sb[:, r * 512:(r + 1) * 512])
                first, last = kt == 0, kt == kt_lim - 1
                nc.tensor.matmul(acc_r[:], onec_sb[:], at[:],
                                 start=first, stop=last)
                nc.tensor.matmul(
                    acc_o[:],
                    V[:, kt * HG + h * 128: kt * HG + h * 128 + 128],
                    at[:], start=first, stop=last)
            # normalize: O^T[:, i] /= rowsum[i]
            rs = fin_pool.tile([1, 512], F32, tag="rs")
            nc.vector.reciprocal(rs[:], acc_r[:])
            bc = ps_b.tile([128, 512], F32, tag="ps_b")
            nc.tensor.matmul(bc[:], oner_sb[:], rs[:], start=True, stop=True)
            rcp = fin_pool.tile([128, 512], F32, tag="rcp")
            nc.scalar.copy(rcp[:], bc[:])
            nc.vector.tensor_mul(
                OT[:, hS + q0: hS + q0 + 512], acc_o[:], rcp[:])


def out_proj(nc, tc, wp_t, OT, out):
    """Phase 3: out_partial = O @ Wp_shard, written straight to DRAM."""
    with tc.tile_pool(name="outst", bufs=4) as outst, \
         tc.tile_pool(name="ps3", bufs=4, space="PSUM") as ps3:
        for ms in range(NST):
            for nc2 in range(NQC):
                acc = ps3.tile([128, 512], F32, tag="ps3")
                for h in range(NH):
                    nc.tensor.matmul(
                        acc[:],
                        OT[:, h * S + ms * 128: h * S + ms * 128 + 128],
                        wp_t[h][:, nc2 * 512:(nc2 + 1) * 512],
                        start=(h == 0), stop=(h == NH - 1),
                    )
                ot = outst.tile([128, 512], F32, tag="outst")
                nc.scalar.copy(ot[:], acc[:])
                nc.sync.dma_start(
                    out[ms * 128:(ms + 1) * 128,
                        nc2 * 512:(nc2 + 1) * 512], ot[:])


def emit_all(nc, tc, xT, wq, wk, wv, wp, out, bqk_sb, bv_sb, mask_sb,
             onec_sb, oner_sb, QT, KT, V, OT):
    qkv_proj(nc, tc, xT, wq, wk, wv, bqk_sb, bv_sb, QT, KT, V)
    with tc.tile_pool(name="wp_pool", bufs=1) as wp_pool:
        wp_t = []
        for h in range(NH):
            t = wp_pool.tile([128, D], BF16, tag=f"wp{h}")
            nc.sync.dma_start(t[:], wp[h * 128:(h + 1) * 128, :])
            wp_t.append(t)
        with tc.tile_pool(name="attn", bufs=4) as attn_pool, \
             tc.tile_pool(name="fin", bufs=2) as fin_pool, \
             tc.tile_pool(name="ps_s", bufs=3, space="PSUM") as ps_s, \
             tc.tile_pool(name="ps_o", bufs=2, space="PSUM") as ps_o, \
             tc.tile_pool(name="ps_r", bufs=2, space="PSUM") as ps_r, \
             tc.tile_pool(name="ps_b", bufs=1, space="PSUM") as ps_b:
            self_attn(nc, tc, attn_pool, fin_pool, ps_s, ps_o, ps_r,
                      ps_b, QT, KT, V, OT, mask_sb, onec_sb, oner_sb)
        out_proj(nc, tc, wp_t, OT, out)


def build(loop_n=1):
    nc = bass.Bass()

    xT = nc.declare_dram_parameter("xT", [D, S], BF16, isOutput=False)
    wq = nc.declare_dram_parameter("wq", [D, HG], BF16, isOutput=False)
    wk = nc.declare_dram_parameter("wk", [D, HG], BF16, isOutput=False)
    wv = nc.declare_dram_parameter("wv", [D, HG], BF16, isOutput=False)
    wp = nc.declare_dram_parameter("wp", [HG, D], BF16, isOutput=False)
    bqk = nc.declare_dram_parameter("bqk", [128, 2 * NH], F32, isOutput=False)
    bv = nc.declare_dram_parameter("bv", [128, HG], F32, isOutput=False)
    masks = nc.declare_dram_parameter("masks", [128, 4 * 512], BF16, isOutput=False)
    ones_col = nc.declare_dram_parameter("ones_col", [128, 1], BF16, isOutput=False)
    ones_row = nc.declare_dram_parameter("ones_row", [1, 128], F32, isOutput=False)
    out = nc.declare_dram_parameter("out", [S, D], F32, isOutput=True)

    with tile.TileContext(nc) as tc:
        with tc.tile_pool(name="const", bufs=1) as cpool, \
             tc.tile_pool(name="qkv", bufs=1) as qkv_pool:
            bqk_sb = cpool.tile([128, 2 * NH], F32, tag="bqk")
            nc.sync.dma_start(bqk_sb[:], bqk[:])
            bv_sb = cpool.tile([128, HG], F32, tag="bv")
            nc.sync.dma_start(bv_sb[:], bv[:])
            mask_sb = cpool.tile([128, 4 * 512], BF16, tag="masks")
            nc.sync.dma_start(mask_sb[:], masks[:])
            onec_sb = cpool.tile([128, 1], BF16, tag="onec")
            nc.sync.dma_start(onec_sb[:], ones_col[:])
            oner_sb = cpool.tile([1, 128], F32, tag="oner")
            nc.sync.dma_start(oner_sb[:], ones_row[:])

            # Per-head feature-major Q^T/K^T/O^T: head h lives in cols
            # [h*S, (h+1)*S). V is token-major: s-tile st in cols
            # [st*HG, (st+1)*HG).
            QT = qkv_pool.tile([128, NH * S], BF16, tag="QT")
            KT = qkv_pool.tile([128, NH * S], BF16, tag="KT")
            V = qkv_pool.tile([128, NST * HG], BF16, tag="V")
            OT = qkv_pool.tile([128, NH * S], BF16, tag="OT")

            if loop_n == 1:
                emit_all(nc, tc, xT, wq, wk, wv, wp, out, bqk_sb, bv_sb,
                         mask_sb, onec_sb, oner_sb, QT, KT, V, OT)
            else:
                with tc.For_i(0, loop_n, 1) as _i:
                    emit_all(nc, tc, xT, wq, wk, wv, wp, out, bqk_sb, bv_sb,
                             mask_sb, onec_sb, oner_sb, QT, KT, V, OT)
    split_excess_waits(nc)
    return nc


_NC_CACHE = {}


def _get_nc(loop_n=1):
    if loop_n not in _NC_CACHE:
        _NC_CACHE[loop_n] = build(loop_n)
    return _NC_CACHE[loop_n]


def _prep_in_maps(x, Wq, bq, Wk, bk, Wv, bv, Wp, bp):
    x = np.asarray(x, dtype=np.float32)
    bf = ml_dtypes.bfloat16
    # causal staircase masks: mask_r[j, i] = 1 if i >= j + r*128
    jj = np.arange(128)[:, None]
    ii = np.arange(512)[None, :]
    masks = np.concatenate(
        [(ii >= jj + r * 128).astype(np.float32) for r in range(4)], axis=1
    ).astype(bf)
    ones_col = np.ones((128, 1), dtype=bf)
    ones_row = np.ones((1, 128), dtype=np.float32)

    xTb = [np.ascontiguousarray(x[b].T).astype(bf) for b in range(B)]
    in_maps = []
    for c in range(8):
        b, g = divmod(c, 4)
        sl = slice(g * HG, (g + 1) * HG)
        bqk = np.concatenate(
            [np.asarray(bq)[sl].reshape(NH, 128).T,
             np.asarray(bk)[sl].reshape(NH, 128).T], axis=1
        ).astype(np.float32)
        bv_rep = np.broadcast_to(
            np.asarray(bv)[sl].astype(np.float32), (128, HG)).copy()
        in_maps.append({
            "xT": xTb[b],
            "wq": np.ascontiguousarray(np.asarray(Wq)[:, sl]).astype(bf),
            "wk": np.ascontiguousarray(np.asarray(Wk)[:, sl]).astype(bf),
            "wv": np.ascontiguousarray(np.asarray(Wv)[:, sl]).astype(bf),
            "wp": np.ascontiguousarray(np.asarray(Wp)[sl, :]).astype(bf),
            "bqk": bqk,
            "bv": bv_rep,
            "masks": masks,
            "ones_col": ones_col,
            "ones_row": ones_row,
        })
    return in_maps


def kernel(x, Wq, bq, Wk, bk, Wv, bv, Wp, bp):
    global LAST_EXEC_NS
    nc = _get_nc()
    in_maps = _prep_in_maps(x, Wq, bq, Wk, bk, Wv, bv, Wp, bp)
    res = run_bass_kernel_spmd(nc, in_maps, core_ids=list(range(8)))
    LAST_EXEC_NS = res.exec_time_ns
    out = np.empty((B, S, D), dtype=np.float32)
    for b in range(B):
        acc = res.results[4 * b]["out"].astype(np.float32)
        for g in range(1, 4):
            acc = acc + res.results[4 * b + g]["out"]
        out[b] = acc
    out += np.asarray(bp, dtype=np.float32)[None, None, :]
    return out


def _make_runner(nc, in_maps):
    """Replicate bass2jax.run_bass_via_pjrt's shard_map jit, returning a
    zero-arg callable over device-resident inputs (for repeat timing)."""
    import jax
    from jax.sharding import Mesh, PartitionSpec, NamedSharding
    from jax.experimental.shard_map import shard_map
    from concourse import bass2jax, mybir as _mybir
    from concourse.bass2jax import _bass_exec_p, install_neuronx_cc_hook

    install_neuronx_cc_hook()
    n_cores = len(in_maps)
    partition_name = (nc.partition_id_tensor.name
                      if nc.partition_id_tensor else None)
    in_names, out_names, out_avals, zero_outs = [], [], [], []
    for alloc in nc.m.functions[0].allocations:
        if not isinstance(alloc, _mybir.MemoryLocationSet):
            continue
        name = alloc.memorylocations[0].name
        if alloc.kind == "ExternalInput":
            if name != partition_name:
                in_names.append(name)
        elif alloc.kind == "ExternalOutput":
            out_names.append(name)
            shape = tuple(alloc.tensor_shape)
            dtype = _mybir.dt.np(alloc.dtype)
            out_avals.append(jax.core.ShapedArray(shape, dtype))
            zero_outs.append(np.zeros(shape, dtype))
    n_params = len(in_names)
    n_outs = len(out_avals)
    in_names = in_names + out_names
    if partition_name is not None:
        in_names.append(partition_name)

    def _body(*args):
        operands = list(args)
        if partition_name is not None:
            operands.append(bass2jax.partition_id_tensor())
        outs = _bass_exec_p.bind(
            *operands, out_avals=tuple(out_avals), in_names=tuple(in_names),
            out_names=tuple(out_names), lowering_input_output_aliases=(),
            sim_require_finite=True, sim_require_nnan=True, nc=nc)
        return tuple(outs)

    devices = jax.devices()[:n_cores]
    mesh = Mesh(np.asarray(devices), ("core",))
    in_specs = (PartitionSpec("core"),) * (n_params + n_outs)
    out_specs = (PartitionSpec("core"),) * len(out_names)
    fn = jax.jit(
        shard_map(_body, mesh=mesh, in_specs=in_specs, out_specs=out_specs,
                  check_rep=False),
        keep_unused=True)
    sh = NamedSharding(mesh, PartitionSpec("core"))
    concat_in = [
        jax.device_put(
            np.concatenate([np.asarray(in_maps[c][in_names[i]])
                            for c in range(n_cores)], axis=0), sh)
        for i in range(n_params)
    ]
    concat_zeros = [
        jax.device_put(np.zeros((n_cores * z.shape[0], *z.shape[1:]), z.dtype), sh)
        for z in zero_outs
    ]
    args = concat_in + concat_zeros

    def run():
        return fn(*args)

    return run


def _time_runner(run, iters):
    import time
    import jax
    jax.block_until_ready(run())  # compile + warm
    times = []
    for _ in range(iters):
        t0 = time.perf_counter()
        jax.block_until_ready(run())
        times.append(time.perf_counter() - t0)
    times.sort()
    return times


def benchmark(inputs, iters=12, loop_n=32):
    """Estimate per-execution HW time by amplifying the kernel body with an
    on-device For_i loop: t = (wall(loop_n) - wall(1)) / (loop_n - 1).
    Tunnel RPC overhead (~100 ms) cancels in the difference."""
    in_maps = _prep_in_maps(**inputs)
    run1 = _make_runner(_get_nc(1), in_maps)
    runN = _make_runner(_get_nc(loop_n), in_maps)
    t1 = _time_runner(run1, iters)
    tN = _time_runner(runN, iters)
    med1 = t1[len(t1) // 2]
    medN = tN[len(tN) // 2]
    est = (medN - med1) / (loop_n - 1)
    print(f"benchmark: wall(1) med {med1*1e3:.1f} ms, wall({loop_n}) med "
          f"{medN*1e3:.1f} ms -> est {est*1e6:.0f} us/exec")
    return est * 1e9


# revision 21
# speedup vs baseline: 171.5218x; 1.0987x over previous
"""Masked causal self-attention on 8 Trainium2 NeuronCores.

Sharding (Megatron-style): core c -> (batch b = c//4, head-group g = c%4).
Each core computes QKV projections for its 4 heads (512 of 2048 cols,
column-parallel), causal attention for those heads on its batch, and a
row-parallel slice of the output projection, producing a partial [S, D]
output. Host sums the 4 partials per batch and adds bp.

On-chip dataflow is fully transposed (feature-major) so no transposes are
ever needed:
  x^T (host-prepped)  --W as lhsT-->  Q^T, K^T [hd, S];  x^T as lhsT --> V [S, hd]
  S^T = (K^T tile).T @ Q^T            [Sk part, Sq free]
  attn^T = exp(S^T * scale) * mask    (no max subtraction: |scores| < ~1)
  rowsum = ones.T @ attn^T            (PE, M=1)
  O^T += (V tile).T @ attn^T          [hd part, Sq free]
  out_partial = (O^T tile).T @ Wp     [S part, D free]
Matmuls in bf16 (4x fp32 TensorE throughput), fp32 PSUM accumulation,
fully masked causal blocks skipped.
"""

import os
import sys

import numpy as np

try:
    import concourse.bass as bass
except ImportError:
    sys.path.insert(0, "/opt/trn_rl_repo")
    import concourse.bass as bass

import ml_dtypes
import concourse.mybir as mybir
import concourse.tile as tile
from concourse.bass_utils import run_bass_kernel_spmd

BF16 = mybir.dt.bfloat16
F32 = mybir.dt.float32
AF = mybir.ActivationFunctionType

B, S, D, H, HD = 2, 2048, 2048, 16, 128
NH = 4                # heads per core
HG = NH * HD          # 512: head-group width per core
NKT = D // 128        # 16 contraction k-tiles over D
NST = S // 128        # 16 s-tiles of 128
NQC = S // 512        # 4 q-chunks of 512
SCALE = 1.0 / float(np.sqrt(D))

LAST_EXEC_NS = None


def split_excess_waits(nc, maxw=1):
    """Walrus in this toolchain rejects >1 sync wait on CTRL-class
    instructions (Tile's tail drain can carry many). Hoist excess waits
    onto preceding single-wait EventSemaphore instructions."""
    for f in nc.m.functions:
        for bb in f.blocks:
            out, changed, k = [], False, 0
            for inst in bb.instructions:
                si = inst.sync_info
                if si is not None and len(si.on_wait) > maxw:
                    waits = list(si.on_wait)
                    while len(waits) > maxw:
                        chunk, waits = waits[:maxw], waits[maxw:]
                        out.append(mybir.InstEventSemaphore(
                            name=f"{inst.name}-waitsplit{k}", engine=inst.engine,
                            sync_info=mybir.SyncInfo(on_wait=chunk, on_update=[])))
                        k += 1
                        changed = True
                    si.on_wait = waits
                out.append(inst)
            if changed:
                bb.instructions = out


def qkv_proj(nc, tc, xT, wq, wk, wv, bqk_sb, bv_sb, QT, KT, V):
    """Phase 1: Q^T,K^T (feature-major) and V (token-major) projections."""
    with tc.tile_pool(name="xw", bufs=1) as xw_pool, \
         tc.tile_pool(name="ps1", bufs=8, space="PSUM") as ps1:
        xt_t = []
        for kt in range(NKT):
            t = xw_pool.tile([128, S], BF16, tag=f"xt{kt}")
            nc.sync.dma_start(t[:], xT[kt * 128:(kt + 1) * 128, :])
            xt_t.append(t)
        w_t = {}
        for nm, dram in (("q", wq), ("k", wk), ("v", wv)):
            for kt in range(NKT):
                t = xw_pool.tile([128, HG], BF16, tag=f"w{nm}{kt}")
                nc.sync.dma_start(t[:], dram[kt * 128:(kt + 1) * 128, :])
                w_t[nm, kt] = t

        # Q^T and K^T: [hd' m-tile 128][Sq chunk 512] = W.T @ x^T.
        # nq inner with a shared lhsT so LDWEIGHTS amortizes over 4 matmuls.
        for nm, dstT, bcol in (("q", QT, 0), ("k", KT, NH)):
            for m in range(NH):
                accs = [ps1.tile([128, 512], F32, tag="ps1",
                                 name=f"acc{i}") for i in range(NQC)]
                for kt in range(NKT):
                    for nq in range(NQC):
                        nc.tensor.matmul(
                            accs[nq][:],
                            w_t[nm, kt][:, m * 128:(m + 1) * 128],
                            xt_t[kt][:, nq * 512:(nq + 1) * 512],
                            start=(kt == 0), stop=(kt == NKT - 1),
                        )
                for nq in range(NQC):
                    nc.scalar.activation(
                        dstT[:, m * S + nq * 512: m * S + nq * 512 + 512],
                        accs[nq][:], AF.Identity,
                        bias=bqk_sb[:, bcol + m: bcol + m + 1],
                    )
        # V (token-major): x^T tile as lhsT
        for st in range(NST):
            acc = ps1.tile([128, 512], F32, tag="ps1")
            for kt in range(NKT):
                nc.tensor.matmul(
                    acc[:],
                    xt_t[kt][:, st * 128:(st + 1) * 128],
                    w_t["v", kt][:],
                    start=(kt == 0), stop=(kt == NKT - 1),
                )
            nc.vector.tensor_add(V[:, st * HG:(st + 1) * HG], acc[:], bv_sb[:])


def self_attn(nc, tc, attn_pool, fin_pool, ps_s, ps_o, ps_r, ps_b,
              QT, KT, V, OT, mask_sb, onec_sb, oner_sb):
    """Phase 2: causal attention per head, transposed-scores flash style."""
    for h in range(NH):
        hS = h * S
        for qc in range(NQC):
            q0 = qc * 512
            kt_lim = 4 * (qc + 1)
            acc_o = ps_o.tile([128, 512], F32, tag="ps_o")
            acc_r = ps_r.tile([1, 512], F32, tag="ps_r")
            for kt in range(kt_lim):
                r = kt - 4 * qc
                ps = ps_s.tile([128, 512], F32, tag="ps_s")
                nc.tensor.matmul(
                    ps[:],
                    KT[:, hS + kt * 128: hS + kt * 128 + 128],
                    QT[:, hS + q0: hS + q0 + 512],
                    start=True, stop=True,
                )
                at = attn_pool.tile([128, 512], BF16, tag="at")
                nc.scalar.activation(at[:], ps[:], AF.Exp, scale=SCALE)
                if r >= 0:  # staircase block: apply causal mask
                    nc.vector.tensor_mul(
                        at[:], at[:], mask_sb[:, r * 512:(r + 1) * 512])
                first, last = kt == 0, kt == kt_lim - 1
                nc.tensor.matmul(acc_r[:], onec_sb[:], at[:],
                                 start=first, stop=last)
                nc.tensor.matmul(
                    acc_o[:],
                    V[:, kt * HG + h * 128: kt * HG + h * 128 + 128],
                    at[:], start=first, stop=last)
            # normalize: O^T[:, i] /= rowsum[i]
            rs = fin_pool.tile([1, 512], F32, tag="rs")
            nc.vector.reciprocal(rs[:], acc_r[:])
            rsb = fin_pool.tile([1, 512], BF16, tag="rsb")
            nc.vector.tensor_copy(rsb[:], rs[:])
            bc = ps_b.tile([128, 512], F32, tag="ps_b")
            nc.tensor.matmul(bc[:], oner_sb[:], rsb[:], start=True, stop=True)
            rcp = fin_pool.tile([128, 512], F32, tag="rcp")
            nc.scalar.copy(rcp[:], bc[:])
            nc.vector.tensor_mul(
                OT[:, hS + q0: hS + q0 + 512], acc_o[:], rcp[:])


def out_proj(nc, tc, wp_t, OT, out):
    """Phase 3: out_partial = O @ Wp_shard, written straight to DRAM."""
    with tc.tile_pool(name="outst", bufs=4) as outst, \
         tc.tile_pool(name="ps3", bufs=8, space="PSUM") as ps3:
        for ms in range(NST):
            accs = [ps3.tile([128, 512], F32, tag="ps3", name=f"acc{i}")
                    for i in range(NQC)]
            for h in range(NH):  # nc2 inner: shared lhsT amortizes LDWEIGHTS
                for nc2 in range(NQC):
                    nc.tensor.matmul(
                        accs[nc2][:],
                        OT[:, h * S + ms * 128: h * S + ms * 128 + 128],
                        wp_t[h][:, nc2 * 512:(nc2 + 1) * 512],
                        start=(h == 0), stop=(h == NH - 1),
                    )
            for nc2 in range(NQC):
                ot = outst.tile([128, 512], F32, tag="outst")
                nc.scalar.copy(ot[:], accs[nc2][:])
                nc.sync.dma_start(
                    out[ms * 128:(ms + 1) * 128,
                        nc2 * 512:(nc2 + 1) * 512], ot[:])


def emit_all(nc, tc, xT, wq, wk, wv, wp, out, bqk_sb, bv_sb, mask_sb,
             onec_sb, oner_sb, QT, KT, V, OT):
    qkv_proj(nc, tc, xT, wq, wk, wv, bqk_sb, bv_sb, QT, KT, V)
    with tc.tile_pool(name="wp_pool", bufs=1) as wp_pool:
        wp_t = []
        for h in range(NH):
            t = wp_pool.tile([128, D], BF16, tag=f"wp{h}")
            nc.sync.dma_start(t[:], wp[h * 128:(h + 1) * 128, :])
            wp_t.append(t)
        with tc.tile_pool(name="attn", bufs=4) as attn_pool, \
             tc.tile_pool(name="fin", bufs=2) as fin_pool, \
             tc.tile_pool(name="ps_s", bufs=3, space="PSUM") as ps_s, \
             tc.tile_pool(name="ps_o", bufs=2, space="PSUM") as ps_o, \
             tc.tile_pool(name="ps_r", bufs=2, space="PSUM") as ps_r, \
             tc.tile_pool(name="ps_b", bufs=1, space="PSUM") as ps_b:
            self_attn(nc, tc, attn_pool, fin_pool, ps_s, ps_o, ps_r,
                      ps_b, QT, KT, V, OT, mask_sb, onec_sb, oner_sb)
        out_proj(nc, tc, wp_t, OT, out)


def build(loop_n=1):
    nc = bass.Bass()

    xT = nc.declare_dram_parameter("xT", [D, S], BF16, isOutput=False)
    wq = nc.declare_dram_parameter("wq", [D, HG], BF16, isOutput=False)
    wk = nc.declare_dram_parameter("wk", [D, HG], BF16, isOutput=False)
    wv = nc.declare_dram_parameter("wv", [D, HG], BF16, isOutput=False)
    wp = nc.declare_dram_parameter("wp", [HG, D], BF16, isOutput=False)
    bqk = nc.declare_dram_parameter("bqk", [128, 2 * NH], F32, isOutput=False)
    bv = nc.declare_dram_parameter("bv", [128, HG], F32, isOutput=False)
    masks = nc.declare_dram_parameter("masks", [128, 4 * 512], BF16, isOutput=False)
    ones_col = nc.declare_dram_parameter("ones_col", [128, 1], BF16, isOutput=False)
    ones_row = nc.declare_dram_parameter("ones_row", [1, 128], BF16, isOutput=False)
    out = nc.declare_dram_parameter("out", [S, D], F32, isOutput=True)

    with tile.TileContext(nc) as tc:
        with tc.tile_pool(name="const", bufs=1) as cpool, \
             tc.tile_pool(name="qkv", bufs=1) as qkv_pool:
            bqk_sb = cpool.tile([128, 2 * NH], F32, tag="bqk")
            nc.sync.dma_start(bqk_sb[:], bqk[:])
            bv_sb = cpool.tile([128, HG], F32, tag="bv")
            nc.sync.dma_start(bv_sb[:], bv[:])
            mask_sb = cpool.tile([128, 4 * 512], BF16, tag="masks")
            nc.sync.dma_start(mask_sb[:], masks[:])
            onec_sb = cpool.tile([128, 1], BF16, tag="onec")
            nc.sync.dma_start(onec_sb[:], ones_col[:])
            oner_sb = cpool.tile([1, 128], BF16, tag="oner")
            nc.sync.dma_start(oner_sb[:], ones_row[:])

            # Per-head feature-major Q^T/K^T/O^T: head h lives in cols
            # [h*S, (h+1)*S). V is token-major: s-tile st in cols
            # [st*HG, (st+1)*HG).
            QT = qkv_pool.tile([128, NH * S], BF16, tag="QT")
            KT = qkv_pool.tile([128, NH * S], BF16, tag="KT")
            V = qkv_pool.tile([128, NST * HG], BF16, tag="V")
            OT = qkv_pool.tile([128, NH * S], BF16, tag="OT")

            if loop_n == 1:
                emit_all(nc, tc, xT, wq, wk, wv, wp, out, bqk_sb, bv_sb,
                         mask_sb, onec_sb, oner_sb, QT, KT, V, OT)
            else:
                with tc.For_i(0, loop_n, 1) as _i:
                    emit_all(nc, tc, xT, wq, wk, wv, wp, out, bqk_sb, bv_sb,
                             mask_sb, onec_sb, oner_sb, QT, KT, V, OT)
    split_excess_waits(nc)
    return nc


_NC_CACHE = {}


def _get_nc(loop_n=1):
    if loop_n not in _NC_CACHE:
        _NC_CACHE[loop_n] = build(loop_n)
    return _NC_CACHE[loop_n]


def _prep_in_maps(x, Wq, bq, Wk, bk, Wv, bv, Wp, bp):
    x = np.asarray(x, dtype=np.float32)
    bf = ml_dtypes.bfloat16
    # causal staircase masks: mask_r[j, i] = 1 if i >= j + r*128
    jj = np.arange(128)[:, None]
    ii = np.arange(512)[None, :]
    masks = np.concatenate(
        [(ii >= jj + r * 128).astype(np.float32) for r in range(4)], axis=1
    ).astype(bf)
    ones_col = np.ones((128, 1), dtype=bf)
    ones_row = np.ones((1, 128), dtype=bf)

    xTb = [np.ascontiguousarray(x[b].T).astype(bf) for b in range(B)]
    in_maps = []
    for c in range(8):
        b, g = divmod(c, 4)
        sl = slice(g * HG, (g + 1) * HG)
        bqk = np.concatenate(
            [np.asarray(bq)[sl].reshape(NH, 128).T,
             np.asarray(bk)[sl].reshape(NH, 128).T], axis=1
        ).astype(np.float32)
        bv_rep = np.broadcast_to(
            np.asarray(bv)[sl].astype(np.float32), (128, HG)).copy()
        in_maps.append({
            "xT": xTb[b],
            "wq": np.ascontiguousarray(np.asarray(Wq)[:, sl]).astype(bf),
            "wk": np.ascontiguousarray(np.asarray(Wk)[:, sl]).astype(bf),
            "wv": np.ascontiguousarray(np.asarray(Wv)[:, sl]).astype(bf),
            "wp": np.ascontiguousarray(np.asarray(Wp)[sl, :]).astype(bf),
            "bqk": bqk,
            "bv": bv_rep,
            "masks": masks,
            "ones_col": ones_col,
            "ones_row": ones_row,
        })
    return in_maps


def kernel(x, Wq, bq, Wk, bk, Wv, bv, Wp, bp):
    global LAST_EXEC_NS
    # NTFF tracing needs antenv.axon_hooks, absent in this container; a set
    # BASS_TRACE would crash run_bass_kernel_spmd otherwise.
    os.environ["BASS_NEVER_TRACE"] = "1"
    nc = _get_nc()
    in_maps = _prep_in_maps(x, Wq, bq, Wk, bk, Wv, bv, Wp, bp)
    res = run_bass_kernel_spmd(nc, in_maps, core_ids=list(range(8)))
    LAST_EXEC_NS = res.exec_time_ns
    out = np.empty((B, S, D), dtype=np.float32)
    for b in range(B):
        acc = res.results[4 * b]["out"].astype(np.float32)
        for g in range(1, 4):
            acc = acc + res.results[4 * b + g]["out"]
        out[b] = acc
    out += np.asarray(bp, dtype=np.float32)[None, None, :]
    return out


def _make_runner(nc, in_maps):
    """Replicate bass2jax.run_bass_via_pjrt's shard_map jit, returning a
    zero-arg callable over device-resident inputs (for repeat timing)."""
    import jax
    from jax.sharding import Mesh, PartitionSpec, NamedSharding
    from jax.experimental.shard_map import shard_map
    from concourse import bass2jax, mybir as _mybir
    from concourse.bass2jax import _bass_exec_p, install_neuronx_cc_hook

    install_neuronx_cc_hook()
    n_cores = len(in_maps)
    partition_name = (nc.partition_id_tensor.name
                      if nc.partition_id_tensor else None)
    in_names, out_names, out_avals, zero_outs = [], [], [], []
    for alloc in nc.m.functions[0].allocations:
        if not isinstance(alloc, _mybir.MemoryLocationSet):
            continue
        name = alloc.memorylocations[0].name
        if alloc.kind == "ExternalInput":
            if name != partition_name:
                in_names.append(name)
        elif alloc.kind == "ExternalOutput":
            out_names.append(name)
            shape = tuple(alloc.tensor_shape)
            dtype = _mybir.dt.np(alloc.dtype)
            out_avals.append(jax.core.ShapedArray(shape, dtype))
            zero_outs.append(np.zeros(shape, dtype))
    n_params = len(in_names)
    n_outs = len(out_avals)
    in_names = in_names + out_names
    if partition_name is not None:
        in_names.append(partition_name)

    def _body(*args):
        operands = list(args)
        if partition_name is not None:
            operands.append(bass2jax.partition_id_tensor())
        outs = _bass_exec_p.bind(
            *operands, out_avals=tuple(out_avals), in_names=tuple(in_names),
            out_names=tuple(out_names), lowering_input_output_aliases=(),
            sim_require_finite=True, sim_require_nnan=True, nc=nc)
        return tuple(outs)

    devices = jax.devices()[:n_cores]
    mesh = Mesh(np.asarray(devices), ("core",))
    in_specs = (PartitionSpec("core"),) * (n_params + n_outs)
    out_specs = (PartitionSpec("core"),) * len(out_names)
    fn = jax.jit(
        shard_map(_body, mesh=mesh, in_specs=in_specs, out_specs=out_specs,
                  check_rep=False),
        keep_unused=True)
    sh = NamedSharding(mesh, PartitionSpec("core"))
    concat_in = [
        jax.device_put(
            np.concatenate([np.asarray(in_maps[c][in_names[i]])
                            for c in range(n_cores)], axis=0), sh)
        for i in range(n_params)
    ]
    concat_zeros = [
        jax.device_put(np.zeros((n_cores * z.shape[0], *z.shape[1:]), z.dtype), sh)
        for z in zero_outs
    ]
    args = concat_in + concat_zeros

    def run():
        return fn(*args)

    return run


def _time_runner(run, iters):
    import time
    import jax
    jax.block_until_ready(run())  # compile + warm
    times = []
    for _ in range(iters):
        t0 = time.perf_counter()
        jax.block_until_ready(run())
        times.append(time.perf_counter() - t0)
    times.sort()
    return times


def benchmark(inputs, iters=12, loop_n=32):
    """Estimate per-execution HW time by amplifying the kernel body with an
    on-device For_i loop: t = (wall(loop_n) - wall(1)) / (loop_n - 1).
    Tunnel RPC overhead (~100 ms) cancels in the difference."""
    in_maps = _prep_in_maps(**inputs)
    run1 = _make_runner(_get_nc(1), in_maps)
    runN = _make_runner(_get_nc(loop_n), in_maps)
    t1 = _time_runner(run1, iters)
    tN = _time_runner(runN, iters)
    med1 = t1[len(t1) // 2]
    medN = tN[len(tN) // 2]
    est = (medN - med1) / (loop_n - 1)
    print(f"benchmark: wall(1) med {med1*1e3:.1f} ms, wall({loop_n}) med "
          f"{medN*1e3:.1f} ms -> est {est*1e6:.0f} us/exec")
    return est * 1e9


# revision 24
# speedup vs baseline: 172.3595x; 1.0049x over previous
"""Masked causal self-attention on 8 Trainium2 NeuronCores.

Sharding (Megatron-style): core c -> (batch b = c//4, head-group g = c%4).
Each core computes QKV projections for its 4 heads (512 of 2048 cols,
column-parallel), causal attention for those heads on its batch, and a
row-parallel slice of the output projection, producing a partial [S, D]
output. Host sums the 4 partials per batch and adds bp.

On-chip dataflow is fully transposed (feature-major) so no transposes are
ever needed:
  x^T (host-prepped)  --W as lhsT-->  Q^T, K^T [hd, S];  x^T as lhsT --> V [S, hd]
  S^T = (K^T tile).T @ Q^T            [Sk part, Sq free]
  attn^T = exp(S^T * scale) * mask    (no max subtraction: |scores| < ~1)
  rowsum = ones.T @ attn^T            (PE, M=1)
  O^T += (V tile).T @ attn^T          [hd part, Sq free]
  out_partial = (O^T tile).T @ Wp     [S part, D free]
Matmuls in bf16 (4x fp32 TensorE throughput), fp32 PSUM accumulation,
fully masked causal blocks skipped.
"""

import os
import sys

import numpy as np

try:
    import concourse.bass as bass
except ImportError:
    sys.path.insert(0, "/opt/trn_rl_repo")
    import concourse.bass as bass

import ml_dtypes
import concourse.mybir as mybir
import concourse.tile as tile
from concourse.bass_utils import run_bass_kernel_spmd

BF16 = mybir.dt.bfloat16
F32 = mybir.dt.float32
AF = mybir.ActivationFunctionType

B, S, D, H, HD = 2, 2048, 2048, 16, 128
NH = 4                # heads per core
HG = NH * HD          # 512: head-group width per core
NKT = D // 128        # 16 contraction k-tiles over D
NST = S // 128        # 16 s-tiles of 128
NQC = S // 512        # 4 q-chunks of 512
SCALE = 1.0 / float(np.sqrt(D))

LAST_EXEC_NS = None


def split_excess_waits(nc, maxw=1):
    """Walrus in this toolchain rejects >1 sync wait on CTRL-class
    instructions (Tile's tail drain can carry many). Hoist excess waits
    onto preceding single-wait EventSemaphore instructions."""
    for f in nc.m.functions:
        for bb in f.blocks:
            out, changed, k = [], False, 0
            for inst in bb.instructions:
                si = inst.sync_info
                if si is not None and len(si.on_wait) > maxw:
                    waits = list(si.on_wait)
                    while len(waits) > maxw:
                        chunk, waits = waits[:maxw], waits[maxw:]
                        out.append(mybir.InstEventSemaphore(
                            name=f"{inst.name}-waitsplit{k}", engine=inst.engine,
                            sync_info=mybir.SyncInfo(on_wait=chunk, on_update=[])))
                        k += 1
                        changed = True
                    si.on_wait = waits
                out.append(inst)
            if changed:
                bb.instructions = out


def qkv_proj(nc, tc, xT, wq, wk, wv, bqk_sb, bv_sb, QT, KT, V):
    """Phase 1: Q^T,K^T (feature-major) and V (token-major) projections."""
    with tc.tile_pool(name="xw", bufs=1) as xw_pool, \
         tc.tile_pool(name="ps1", bufs=8, space="PSUM") as ps1:
        xt_t = []
        for kt in range(NKT):
            t = xw_pool.tile([128, S], BF16, tag=f"xt{kt}")
            nc.sync.dma_start(t[:], xT[kt * 128:(kt + 1) * 128, :])
            xt_t.append(t)
        w_t = {}
        for nm, dram in (("q", wq), ("k", wk), ("v", wv)):
            for kt in range(NKT):
                t = xw_pool.tile([128, HG], BF16, tag=f"w{nm}{kt}")
                nc.sync.dma_start(t[:], dram[kt * 128:(kt + 1) * 128, :])
                w_t[nm, kt] = t

        # Q^T and K^T: [hd' m-tile 128][Sq chunk 512] = W.T @ x^T.
        # Chains grouped 4-wide (1 m-tile x 4 nq) so two groups rotate through
        # the 8 PSUM banks: group g's bias-activations overlap group g+1's
        # matmuls instead of draining PE. Shared lhsT per (m, kt) still
        # amortizes LDWEIGHTS over 4 matmuls.
        for nm, dstT, bcol in (("q", QT, 0), ("k", KT, NH)):
            for m in range(NH):
                accs = [ps1.tile([128, 512], F32, tag="ps1",
                                 name=f"acc{i}") for i in range(NQC)]
                for kt in range(NKT):
                    for nq in range(NQC):
                        nc.tensor.matmul(
                            accs[nq][:],
                            w_t[nm, kt][:, m * 128:(m + 1) * 128],
                            xt_t[kt][:, nq * 512:(nq + 1) * 512],
                            start=(kt == 0), stop=(kt == NKT - 1),
                        )
                for nq in range(NQC):
                    nc.scalar.activation(
                        dstT[:, m * S + nq * 512: m * S + nq * 512 + 512],
                        accs[nq][:], AF.Identity,
                        bias=bqk_sb[:, bcol + m: bcol + m + 1],
                    )
        # V (token-major): x^T tile as lhsT, 4 s-tiles per group (2 groups
        # in flight over the 8 banks)
        for sg in range(NST // 4):
            accs = [ps1.tile([128, 512], F32, tag="ps1",
                             name=f"acc{i}") for i in range(4)]
            for kt in range(NKT):
                for si in range(4):
                    st = 4 * sg + si
                    nc.tensor.matmul(
                        accs[si][:],
                        xt_t[kt][:, st * 128:(st + 1) * 128],
                        w_t["v", kt][:],
                        start=(kt == 0), stop=(kt == NKT - 1),
                    )
            for si in range(4):
                st = 4 * sg + si
                nc.vector.tensor_add(V[:, st * HG:(st + 1) * HG],
                                     accs[si][:], bv_sb[:])


def self_attn(nc, tc, attn_pool, fin_pool, ps_s, ps_o, ps_r, ps_b,
              QT, KT, V, OT, mask_sb, onec_sb, oner_sb):
    """Phase 2: causal attention per head, transposed-scores flash style.

    Software-pipelined with lookahead 2: scores(k+2) is emitted before
    O-matmul(k), so while ScalarE exps block k+1 / VectorE masks it, PE
    streams the next scores block instead of stalling. Rowsum is accumulated
    on VectorE (f32 SBUF) with a single [1,512] PE matmul per chain, instead
    of 1 full-cost PE matmul per block."""
    LOOK = 2
    for h in range(NH):
        hS = h * S
        for qc in range(NQC):
            q0 = qc * 512
            kt_lim = 4 * (qc + 1)
            acc_o = ps_o.tile([128, 512], F32, tag="ps_o")
            racc = fin_pool.tile([128, 512], F32, tag="racc")

            def emit_scores(kt):
                r = kt - 4 * qc
                ps = ps_s.tile([128, 512], F32, tag="ps_s")
                nc.tensor.matmul(
                    ps[:],
                    KT[:, hS + kt * 128: hS + kt * 128 + 128],
                    QT[:, hS + q0: hS + q0 + 512],
                    start=True, stop=True,
                )
                at = attn_pool.tile([128, 512], BF16, tag="at")
                nc.scalar.activation(at[:], ps[:], AF.Exp, scale=SCALE)
                if r >= 0:  # staircase block: apply causal mask
                    nc.vector.tensor_mul(
                        at[:], at[:], mask_sb[:, r * 512:(r + 1) * 512])
                return at

            ats = {kt: emit_scores(kt) for kt in range(min(LOOK, kt_lim))}
            for kt in range(kt_lim):
                if kt + LOOK < kt_lim:
                    ats[kt + LOOK] = emit_scores(kt + LOOK)
                at = ats.pop(kt)
                if kt == 0:
                    nc.vector.tensor_copy(racc[:], at[:])
                else:
                    nc.vector.tensor_add(racc[:], racc[:], at[:])
                nc.tensor.matmul(
                    acc_o[:],
                    V[:, kt * HG + h * 128: kt * HG + h * 128 + 128],
                    at[:], start=(kt == 0), stop=(kt == kt_lim - 1))
            # normalize: O^T[:, i] /= rowsum[i]
            rb = fin_pool.tile([128, 512], BF16, tag="rb")
            nc.vector.tensor_copy(rb[:], racc[:])
            acc_r = ps_r.tile([1, 512], F32, tag="ps_r")
            nc.tensor.matmul(acc_r[:], onec_sb[:], rb[:],
                             start=True, stop=True)
            rs = fin_pool.tile([1, 512], F32, tag="rs")
            nc.vector.reciprocal(rs[:], acc_r[:])
            rsb = fin_pool.tile([1, 512], BF16, tag="rsb")
            nc.vector.tensor_copy(rsb[:], rs[:])
            bc = ps_b.tile([128, 512], F32, tag="ps_b")
            nc.tensor.matmul(bc[:], oner_sb[:], rsb[:], start=True, stop=True)
            rcp = fin_pool.tile([128, 512], F32, tag="rcp")
            nc.scalar.copy(rcp[:], bc[:])
            nc.vector.tensor_mul(
                OT[:, hS + q0: hS + q0 + 512], acc_o[:], rcp[:])


def out_proj(nc, tc, wp_t, OT, out):
    """Phase 3: out_partial = O @ Wp_shard, written straight to DRAM."""
    with tc.tile_pool(name="outst", bufs=4) as outst, \
         tc.tile_pool(name="ps3", bufs=8, space="PSUM") as ps3:
        for ms in range(NST):
            accs = [ps3.tile([128, 512], F32, tag="ps3", name=f"acc{i}")
                    for i in range(NQC)]
            for h in range(NH):  # nc2 inner: shared lhsT amortizes LDWEIGHTS
                for nc2 in range(NQC):
                    nc.tensor.matmul(
                        accs[nc2][:],
                        OT[:, h * S + ms * 128: h * S + ms * 128 + 128],
                        wp_t[h][:, nc2 * 512:(nc2 + 1) * 512],
                        start=(h == 0), stop=(h == NH - 1),
                    )
            for nc2 in range(NQC):
                ot = outst.tile([128, 512], F32, tag="outst")
                nc.scalar.copy(ot[:], accs[nc2][:])
                nc.sync.dma_start(
                    out[ms * 128:(ms + 1) * 128,
                        nc2 * 512:(nc2 + 1) * 512], ot[:])


def emit_all(nc, tc, xT, wq, wk, wv, wp, out, bqk_sb, bv_sb, mask_sb,
             onec_sb, oner_sb, QT, KT, V, OT):
    qkv_proj(nc, tc, xT, wq, wk, wv, bqk_sb, bv_sb, QT, KT, V)
    with tc.tile_pool(name="wp_pool", bufs=1) as wp_pool:
        wp_t = []
        for h in range(NH):
            t = wp_pool.tile([128, D], BF16, tag=f"wp{h}")
            nc.sync.dma_start(t[:], wp[h * 128:(h + 1) * 128, :])
            wp_t.append(t)
        with tc.tile_pool(name="attn", bufs=4) as attn_pool, \
             tc.tile_pool(name="fin", bufs=2) as fin_pool, \
             tc.tile_pool(name="ps_s", bufs=3, space="PSUM") as ps_s, \
             tc.tile_pool(name="ps_o", bufs=2, space="PSUM") as ps_o, \
             tc.tile_pool(name="ps_r", bufs=2, space="PSUM") as ps_r, \
             tc.tile_pool(name="ps_b", bufs=1, space="PSUM") as ps_b:
            self_attn(nc, tc, attn_pool, fin_pool, ps_s, ps_o, ps_r,
                      ps_b, QT, KT, V, OT, mask_sb, onec_sb, oner_sb)
        out_proj(nc, tc, wp_t, OT, out)


def build(loop_n=1):
    nc = bass.Bass()

    xT = nc.declare_dram_parameter("xT", [D, S], BF16, isOutput=False)
    wq = nc.declare_dram_parameter("wq", [D, HG], BF16, isOutput=False)
    wk = nc.declare_dram_parameter("wk", [D, HG], BF16, isOutput=False)
    wv = nc.declare_dram_parameter("wv", [D, HG], BF16, isOutput=False)
    wp = nc.declare_dram_parameter("wp", [HG, D], BF16, isOutput=False)
    bqk = nc.declare_dram_parameter("bqk", [128, 2 * NH], F32, isOutput=False)
    bv = nc.declare_dram_parameter("bv", [128, HG], F32, isOutput=False)
    masks = nc.declare_dram_parameter("masks", [128, 4 * 512], BF16, isOutput=False)
    ones_col = nc.declare_dram_parameter("ones_col", [128, 1], BF16, isOutput=False)
    ones_row = nc.declare_dram_parameter("ones_row", [1, 128], BF16, isOutput=False)
    out = nc.declare_dram_parameter("out", [S, D], F32, isOutput=True)

    with tile.TileContext(nc) as tc:
        with tc.tile_pool(name="const", bufs=1) as cpool, \
             tc.tile_pool(name="qkv", bufs=1) as qkv_pool:
            bqk_sb = cpool.tile([128, 2 * NH], F32, tag="bqk")
            nc.sync.dma_start(bqk_sb[:], bqk[:])
            bv_sb = cpool.tile([128, HG], F32, tag="bv")
            nc.sync.dma_start(bv_sb[:], bv[:])
            mask_sb = cpool.tile([128, 4 * 512], BF16, tag="masks")
            nc.sync.dma_start(mask_sb[:], masks[:])
            onec_sb = cpool.tile([128, 1], BF16, tag="onec")
            nc.sync.dma_start(onec_sb[:], ones_col[:])
            oner_sb = cpool.tile([1, 128], BF16, tag="oner")
            nc.sync.dma_start(oner_sb[:], ones_row[:])

            # Per-head feature-major Q^T/K^T/O^T: head h lives in cols
            # [h*S, (h+1)*S). V is token-major: s-tile st in cols
            # [st*HG, (st+1)*HG).
            QT = qkv_pool.tile([128, NH * S], BF16, tag="QT")
            KT = qkv_pool.tile([128, NH * S], BF16, tag="KT")
            V = qkv_pool.tile([128, NST * HG], BF16, tag="V")
            OT = qkv_pool.tile([128, NH * S], BF16, tag="OT")

            if loop_n == 1:
                emit_all(nc, tc, xT, wq, wk, wv, wp, out, bqk_sb, bv_sb,
                         mask_sb, onec_sb, oner_sb, QT, KT, V, OT)
            else:
                with tc.For_i(0, loop_n, 1) as _i:
                    emit_all(nc, tc, xT, wq, wk, wv, wp, out, bqk_sb, bv_sb,
                             mask_sb, onec_sb, oner_sb, QT, KT, V, OT)
    split_excess_waits(nc)
    return nc


_NC_CACHE = {}


def _get_nc(loop_n=1):
    if loop_n not in _NC_CACHE:
        _NC_CACHE[loop_n] = build(loop_n)
    return _NC_CACHE[loop_n]


def _prep_in_maps(x, Wq, bq, Wk, bk, Wv, bv, Wp, bp):
    x = np.asarray(x, dtype=np.float32)
    bf = ml_dtypes.bfloat16
    # causal staircase masks: mask_r[j, i] = 1 if i >= j + r*128
    jj = np.arange(128)[:, None]
    ii = np.arange(512)[None, :]
    masks = np.concatenate(
        [(ii >= jj + r * 128).astype(np.float32) for r in range(4)], axis=1
    ).astype(bf)
    ones_col = np.ones((128, 1), dtype=bf)
    ones_row = np.ones((1, 128), dtype=bf)

    xTb = [np.ascontiguousarray(x[b].T).astype(bf) for b in range(B)]
    in_maps = []
    for c in range(8):
        b, g = divmod(c, 4)
        sl = slice(g * HG, (g + 1) * HG)
        bqk = np.concatenate(
            [np.asarray(bq)[sl].reshape(NH, 128).T,
             np.asarray(bk)[sl].reshape(NH, 128).T], axis=1
        ).astype(np.float32)
        bv_rep = np.broadcast_to(
            np.asarray(bv)[sl].astype(np.float32), (128, HG)).copy()
        in_maps.append({
            "xT": xTb[b],
            "wq": np.ascontiguousarray(np.asarray(Wq)[:, sl]).astype(bf),
            "wk": np.ascontiguousarray(np.asarray(Wk)[:, sl]).astype(bf),
            "wv": np.ascontiguousarray(np.asarray(Wv)[:, sl]).astype(bf),
            "wp": np.ascontiguousarray(np.asarray(Wp)[sl, :]).astype(bf),
            "bqk": bqk,
            "bv": bv_rep,
            "masks": masks,
            "ones_col": ones_col,
            "ones_row": ones_row,
        })
    return in_maps


def kernel(x, Wq, bq, Wk, bk, Wv, bv, Wp, bp):
    global LAST_EXEC_NS
    # NTFF tracing needs antenv.axon_hooks, absent in this container; a set
    # BASS_TRACE would crash run_bass_kernel_spmd otherwise.
    os.environ["BASS_NEVER_TRACE"] = "1"
    nc = _get_nc()
    in_maps = _prep_in_maps(x, Wq, bq, Wk, bk, Wv, bv, Wp, bp)
    res = run_bass_kernel_spmd(nc, in_maps, core_ids=list(range(8)))
    LAST_EXEC_NS = res.exec_time_ns
    out = np.empty((B, S, D), dtype=np.float32)
    for b in range(B):
        acc = res.results[4 * b]["out"].astype(np.float32)
        for g in range(1, 4):
            acc = acc + res.results[4 * b + g]["out"]
        out[b] = acc
    out += np.asarray(bp, dtype=np.float32)[None, None, :]
    return out


def _make_runner(nc, in_maps):
    """Replicate bass2jax.run_bass_via_pjrt's shard_map jit, returning a
    zero-arg callable over device-resident inputs (for repeat timing)."""
    import jax
    from jax.sharding import Mesh, PartitionSpec, NamedSharding
    from jax.experimental.shard_map import shard_map
    from concourse import bass2jax, mybir as _mybir
    from concourse.bass2jax import _bass_exec_p, install_neuronx_cc_hook

    install_neuronx_cc_hook()
    n_cores = len(in_maps)
    partition_name = (nc.partition_id_tensor.name
                      if nc.partition_id_tensor else None)
    in_names, out_names, out_avals, zero_outs = [], [], [], []
    for alloc in nc.m.functions[0].allocations:
        if not isinstance(alloc, _mybir.MemoryLocationSet):
            continue
        name = alloc.memorylocations[0].name
        if alloc.kind == "ExternalInput":
            if name != partition_name:
                in_names.append(name)
        elif alloc.kind == "ExternalOutput":
            out_names.append(name)
            shape = tuple(alloc.tensor_shape)
            dtype = _mybir.dt.np(alloc.dtype)
            out_avals.append(jax.core.ShapedArray(shape, dtype))
            zero_outs.append(np.zeros(shape, dtype))
    n_params = len(in_names)
    n_outs = len(out_avals)
    in_names = in_names + out_names
    if partition_name is not None:
        in_names.append(partition_name)

    def _body(*args):
        operands = list(args)
        if partition_name is not None:
            operands.append(bass2jax.partition_id_tensor())
        outs = _bass_exec_p.bind(
            *operands, out_avals=tuple(out_avals), in_names=tuple(in_names),
            out_names=tuple(out_names), lowering_input_output_aliases=(),
            sim_require_finite=True, sim_require_nnan=True, nc=nc)
        return tuple(outs)

    devices = jax.devices()[:n_cores]
    mesh = Mesh(np.asarray(devices), ("core",))
    in_specs = (PartitionSpec("core"),) * (n_params + n_outs)
    out_specs = (PartitionSpec("core"),) * len(out_names)
    fn = jax.jit(
        shard_map(_body, mesh=mesh, in_specs=in_specs, out_specs=out_specs,
                  check_rep=False),
        keep_unused=True)
    sh = NamedSharding(mesh, PartitionSpec("core"))
    concat_in = [
        jax.device_put(
            np.concatenate([np.asarray(in_maps[c][in_names[i]])
                            for c in range(n_cores)], axis=0), sh)
        for i in range(n_params)
    ]
    concat_zeros = [
        jax.device_put(np.zeros((n_cores * z.shape[0], *z.shape[1:]), z.dtype), sh)
        for z in zero_outs
    ]
    args = concat_in + concat_zeros

    def run():
        return fn(*args)

    return run


def _time_runner(run, iters):
    import time
    import jax
    jax.block_until_ready(run())  # compile + warm
    times = []
    for _ in range(iters):
        t0 = time.perf_counter()
        jax.block_until_ready(run())
        times.append(time.perf_counter() - t0)
    times.sort()
    return times


def benchmark(inputs, iters=12, loop_n=32):
    """Estimate per-execution HW time by amplifying the kernel body with an
    on-device For_i loop: t = (wall(loop_n) - wall(1)) / (loop_n - 1).
    Tunnel RPC overhead (~100 ms) cancels in the difference."""
    in_maps = _prep_in_maps(**inputs)
    run1 = _make_runner(_get_nc(1), in_maps)
    runN = _make_runner(_get_nc(loop_n), in_maps)
    t1 = _time_runner(run1, iters)
    tN = _time_runner(runN, iters)
    med1 = t1[len(t1) // 2]
    medN = tN[len(tN) // 2]
    est = (medN - med1) / (loop_n - 1)
    print(f"benchmark: wall(1) med {med1*1e3:.1f} ms, wall({loop_n}) med "
          f"{medN*1e3:.1f} ms -> est {est*1e6:.0f} us/exec")
    return est * 1e9



# revision 27
# speedup vs baseline: 173.3085x; 1.0055x over previous
"""Masked causal self-attention on 8 Trainium2 NeuronCores.

Sharding (Megatron-style): core c -> (batch b = c//4, head-group g = c%4).
Each core computes QKV projections for its 4 heads (512 of 2048 cols,
column-parallel), causal attention for those heads on its batch, and a
row-parallel slice of the output projection, producing a partial [S, D]
output. Host sums the 4 partials per batch and adds bp.

On-chip dataflow is fully transposed (feature-major) so no transposes are
ever needed:
  x^T (host-prepped)  --W as lhsT-->  Q^T, K^T [hd, S];  x^T as lhsT --> V [S, hd]
  S^T = (K^T tile).T @ Q^T            [Sk part, Sq free]
  attn^T = exp(S^T * scale) * mask    (no max subtraction: |scores| < ~1)
  rowsum = ones.T @ attn^T            (PE, M=1)
  O^T += (V tile).T @ attn^T          [hd part, Sq free]
  out_partial = (O^T tile).T @ Wp     [S part, D free]
Matmuls in bf16 (4x fp32 TensorE throughput), fp32 PSUM accumulation,
fully masked causal blocks skipped.
"""

import os
import sys

import numpy as np

try:
    import concourse.bass as bass
except ImportError:
    sys.path.insert(0, "/opt/trn_rl_repo")
    import concourse.bass as bass

import ml_dtypes
import concourse.mybir as mybir
import concourse.tile as tile
from concourse.bass_utils import run_bass_kernel_spmd

BF16 = mybir.dt.bfloat16
F32 = mybir.dt.float32
AF = mybir.ActivationFunctionType

B, S, D, H, HD = 2, 2048, 2048, 16, 128
NH = 4                # heads per core
HG = NH * HD          # 512: head-group width per core
NKT = D // 128        # 16 contraction k-tiles over D
NST = S // 128        # 16 s-tiles of 128
NQC = S // 512        # 4 q-chunks of 512
SCALE = 1.0 / float(np.sqrt(D))

LAST_EXEC_NS = None


def split_excess_waits(nc, maxw=1):
    """Walrus in this toolchain rejects >1 sync wait on CTRL-class
    instructions (Tile's tail drain can carry many). Hoist excess waits
    onto preceding single-wait EventSemaphore instructions."""
    for f in nc.m.functions:
        for bb in f.blocks:
            out, changed, k = [], False, 0
            for inst in bb.instructions:
                si = inst.sync_info
                if si is not None and len(si.on_wait) > maxw:
                    waits = list(si.on_wait)
                    while len(waits) > maxw:
                        chunk, waits = waits[:maxw], waits[maxw:]
                        out.append(mybir.InstEventSemaphore(
                            name=f"{inst.name}-waitsplit{k}", engine=inst.engine,
                            sync_info=mybir.SyncInfo(on_wait=chunk, on_update=[])))
                        k += 1
                        changed = True
                    si.on_wait = waits
                out.append(inst)
            if changed:
                bb.instructions = out


def qkv_proj(nc, tc, xT, wq, wk, wv, bqk_sb, bv_sb, QT, KT, V):
    """Phase 1: Q^T,K^T (feature-major) and V (token-major) projections."""
    with tc.tile_pool(name="xw", bufs=1) as xw_pool, \
         tc.tile_pool(name="ps1", bufs=8, space="PSUM") as ps1:
        xt_t = []
        for kt in range(NKT):
            t = xw_pool.tile([128, S], BF16, tag=f"xt{kt}")
            nc.sync.dma_start(t[:], xT[kt * 128:(kt + 1) * 128, :])
            xt_t.append(t)
        w_t = {}
        for nm, dram in (("q", wq), ("k", wk), ("v", wv)):
            for kt in range(NKT):
                t = xw_pool.tile([128, HG], BF16, tag=f"w{nm}{kt}")
                nc.sync.dma_start(t[:], dram[kt * 128:(kt + 1) * 128, :])
                w_t[nm, kt] = t

        # Q^T and K^T: [hd' m-tile 128][Sq chunk 512] = W.T @ x^T.
        # Chains grouped 4-wide (1 m-tile x 4 nq) so two groups rotate through
        # the 8 PSUM banks: group g's bias-activations overlap group g+1's
        # matmuls instead of draining PE. Shared lhsT per (m, kt) still
        # amortizes LDWEIGHTS over 4 matmuls.
        for nm, dstT, bcol in (("q", QT, 0), ("k", KT, NH)):
            for m in range(NH):
                accs = [ps1.tile([128, 512], F32, tag="ps1",
                                 name=f"acc{i}") for i in range(NQC)]
                for kt in range(NKT):
                    for nq in range(NQC):
                        nc.tensor.matmul(
                            accs[nq][:],
                            w_t[nm, kt][:, m * 128:(m + 1) * 128],
                            xt_t[kt][:, nq * 512:(nq + 1) * 512],
                            start=(kt == 0), stop=(kt == NKT - 1),
                        )
                for nq in range(NQC):
                    nc.scalar.activation(
                        dstT[:, m * S + nq * 512: m * S + nq * 512 + 512],
                        accs[nq][:], AF.Identity,
                        bias=bqk_sb[:, bcol + m: bcol + m + 1],
                    )
        # V (token-major): x^T tile as lhsT, 4 s-tiles per group (2 groups
        # in flight over the 8 banks)
        for sg in range(NST // 4):
            accs = [ps1.tile([128, 512], F32, tag="ps1",
                             name=f"acc{i}") for i in range(4)]
            for kt in range(NKT):
                for si in range(4):
                    st = 4 * sg + si
                    nc.tensor.matmul(
                        accs[si][:],
                        xt_t[kt][:, st * 128:(st + 1) * 128],
                        w_t["v", kt][:],
                        start=(kt == 0), stop=(kt == NKT - 1),
                    )
            for si in range(4):
                st = 4 * sg + si
                nc.vector.tensor_add(V[:, st * HG:(st + 1) * HG],
                                     accs[si][:], bv_sb[:])


def self_attn(nc, tc, attn_pool, fin_pool, ps_s, ps_o, ps_r, ps_b,
              QT, KT, V, OT, mask_sb, onec_sb, oner_sb):
    """Phase 2: causal attention per head, transposed-scores flash style.

    Software-pipelined with lookahead 2: scores(k+2) is emitted before
    O-matmul(k), so while ScalarE exps block k+1 / VectorE masks it, PE
    streams the next scores block instead of stalling. Rowsum is accumulated
    on VectorE (f32 SBUF) with a single [1,512] PE matmul per chain, instead
    of 1 full-cost PE matmul per block."""
    LOOK = 2
    for h in range(NH):
        hS = h * S
        for qc in range(NQC):
            q0 = qc * 512
            kt_lim = 4 * (qc + 1)
            acc_o = ps_o.tile([128, 512], F32, tag="ps_o")
            racc = fin_pool.tile([128, 512], F32, tag="racc")

            def emit_scores(kt):
                r = kt - 4 * qc
                ps = ps_s.tile([128, 512], F32, tag="ps_s")
                nc.tensor.matmul(
                    ps[:],
                    KT[:, hS + kt * 128: hS + kt * 128 + 128],
                    QT[:, hS + q0: hS + q0 + 512],
                    start=True, stop=True,
                )
                at = attn_pool.tile([128, 512], BF16, tag="at")
                nc.scalar.activation(at[:], ps[:], AF.Exp, scale=SCALE)
                if r >= 0:  # staircase block: apply causal mask
                    nc.vector.tensor_mul(
                        at[:], at[:], mask_sb[:, r * 512:(r + 1) * 512])
                return at

            ats = {kt: emit_scores(kt) for kt in range(min(LOOK, kt_lim))}
            for kt in range(kt_lim):
                if kt + LOOK < kt_lim:
                    ats[kt + LOOK] = emit_scores(kt + LOOK)
                at = ats.pop(kt)
                if kt == 0:
                    nc.vector.tensor_copy(racc[:], at[:])
                else:
                    nc.vector.tensor_add(racc[:], racc[:], at[:])
                nc.tensor.matmul(
                    acc_o[:],
                    V[:, kt * HG + h * 128: kt * HG + h * 128 + 128],
                    at[:], start=(kt == 0), stop=(kt == kt_lim - 1))
            # normalize: O^T[:, i] /= rowsum[i]
            rb = fin_pool.tile([128, 512], BF16, tag="rb")
            nc.vector.tensor_copy(rb[:], racc[:])
            acc_r = ps_r.tile([1, 512], F32, tag="ps_r")
            nc.tensor.matmul(acc_r[:], onec_sb[:], rb[:],
                             start=True, stop=True)
            rs = fin_pool.tile([1, 512], F32, tag="rs")
            nc.vector.reciprocal(rs[:], acc_r[:])
            rsb = fin_pool.tile([1, 512], BF16, tag="rsb")
            nc.vector.tensor_copy(rsb[:], rs[:])
            bc = ps_b.tile([128, 512], F32, tag="ps_b")
            nc.tensor.matmul(bc[:], oner_sb[:], rsb[:], start=True, stop=True)
            rcp = fin_pool.tile([128, 512], F32, tag="rcp")
            nc.scalar.copy(rcp[:], bc[:])
            nc.vector.tensor_mul(
                OT[:, hS + q0: hS + q0 + 512], acc_o[:], rcp[:])


def out_proj(nc, tc, wp_t, OT, out):
    """Phase 3: out_partial = O @ Wp_shard, written straight to DRAM."""
    with tc.tile_pool(name="outst", bufs=4) as outst, \
         tc.tile_pool(name="ps3", bufs=8, space="PSUM") as ps3:
        for ms in range(NST):
            accs = [ps3.tile([128, 512], F32, tag="ps3", name=f"acc{i}")
                    for i in range(NQC)]
            for h in range(NH):  # nc2 inner: shared lhsT amortizes LDWEIGHTS
                for nc2 in range(NQC):
                    nc.tensor.matmul(
                        accs[nc2][:],
                        OT[:, h * S + ms * 128: h * S + ms * 128 + 128],
                        wp_t[h][:, nc2 * 512:(nc2 + 1) * 512],
                        start=(h == 0), stop=(h == NH - 1),
                    )
            for nc2 in range(NQC):
                # bf16 partial store halves the dominant DMA stream (16->8 MB);
                # host sums the 4 partials per batch in f32.
                ot = outst.tile([128, 512], BF16, tag="outst")
                nc.scalar.copy(ot[:], accs[nc2][:])
                nc.sync.dma_start(
                    out[ms * 128:(ms + 1) * 128,
                        nc2 * 512:(nc2 + 1) * 512], ot[:])


def emit_all(nc, tc, xT, wq, wk, wv, wp, out, bqk_sb, bv_sb, mask_sb,
             onec_sb, oner_sb, QT, KT, V, OT):
    qkv_proj(nc, tc, xT, wq, wk, wv, bqk_sb, bv_sb, QT, KT, V)
    with tc.tile_pool(name="wp_pool", bufs=1) as wp_pool:
        wp_t = []
        for h in range(NH):
            t = wp_pool.tile([128, D], BF16, tag=f"wp{h}")
            nc.sync.dma_start(t[:], wp[h * 128:(h + 1) * 128, :])
            wp_t.append(t)
        with tc.tile_pool(name="attn", bufs=4) as attn_pool, \
             tc.tile_pool(name="fin", bufs=2) as fin_pool, \
             tc.tile_pool(name="ps_s", bufs=3, space="PSUM") as ps_s, \
             tc.tile_pool(name="ps_o", bufs=2, space="PSUM") as ps_o, \
             tc.tile_pool(name="ps_r", bufs=2, space="PSUM") as ps_r, \
             tc.tile_pool(name="ps_b", bufs=1, space="PSUM") as ps_b:
            self_attn(nc, tc, attn_pool, fin_pool, ps_s, ps_o, ps_r,
                      ps_b, QT, KT, V, OT, mask_sb, onec_sb, oner_sb)
        out_proj(nc, tc, wp_t, OT, out)


def build(loop_n=1):
    nc = bass.Bass()

    xT = nc.declare_dram_parameter("xT", [D, S], BF16, isOutput=False)
    wq = nc.declare_dram_parameter("wq", [D, HG], BF16, isOutput=False)
    wk = nc.declare_dram_parameter("wk", [D, HG], BF16, isOutput=False)
    wv = nc.declare_dram_parameter("wv", [D, HG], BF16, isOutput=False)
    wp = nc.declare_dram_parameter("wp", [HG, D], BF16, isOutput=False)
    bqk = nc.declare_dram_parameter("bqk", [128, 2 * NH], F32, isOutput=False)
    bv = nc.declare_dram_parameter("bv", [128, HG], F32, isOutput=False)
    masks = nc.declare_dram_parameter("masks", [128, 4 * 512], BF16, isOutput=False)
    ones_col = nc.declare_dram_parameter("ones_col", [128, 1], BF16, isOutput=False)
    ones_row = nc.declare_dram_parameter("ones_row", [1, 128], BF16, isOutput=False)
    out = nc.declare_dram_parameter("out", [S, D], BF16, isOutput=True)

    with tile.TileContext(nc) as tc:
        with tc.tile_pool(name="const", bufs=1) as cpool, \
             tc.tile_pool(name="qkv", bufs=1) as qkv_pool:
            bqk_sb = cpool.tile([128, 2 * NH], F32, tag="bqk")
            nc.sync.dma_start(bqk_sb[:], bqk[:])
            bv_sb = cpool.tile([128, HG], F32, tag="bv")
            nc.sync.dma_start(bv_sb[:], bv[:])
            mask_sb = cpool.tile([128, 4 * 512], BF16, tag="masks")
            nc.sync.dma_start(mask_sb[:], masks[:])
            onec_sb = cpool.tile([128, 1], BF16, tag="onec")
            nc.sync.dma_start(onec_sb[:], ones_col[:])
            oner_sb = cpool.tile([1, 128], BF16, tag="oner")
            nc.sync.dma_start(oner_sb[:], ones_row[:])

            # Per-head feature-major Q^T/K^T/O^T: head h lives in cols
            # [h*S, (h+1)*S). V is token-major: s-tile st in cols
            # [st*HG, (st+1)*HG).
            QT = qkv_pool.tile([128, NH * S], BF16, tag="QT")
            KT = qkv_pool.tile([128, NH * S], BF16, tag="KT")
            V = qkv_pool.tile([128, NST * HG], BF16, tag="V")
            OT = qkv_pool.tile([128, NH * S], BF16, tag="OT")

            if loop_n == 1:
                emit_all(nc, tc, xT, wq, wk, wv, wp, out, bqk_sb, bv_sb,
                         mask_sb, onec_sb, oner_sb, QT, KT, V, OT)
            else:
                with tc.For_i(0, loop_n, 1) as _i:
                    emit_all(nc, tc, xT, wq, wk, wv, wp, out, bqk_sb, bv_sb,
                             mask_sb, onec_sb, oner_sb, QT, KT, V, OT)
    split_excess_waits(nc)
    return nc


_NC_CACHE = {}


def _get_nc(loop_n=1):
    if loop_n not in _NC_CACHE:
        _NC_CACHE[loop_n] = build(loop_n)
    return _NC_CACHE[loop_n]


def _prep_in_maps(x, Wq, bq, Wk, bk, Wv, bv, Wp, bp):
    x = np.asarray(x, dtype=np.float32)
    bf = ml_dtypes.bfloat16
    # causal staircase masks: mask_r[j, i] = 1 if i >= j + r*128
    jj = np.arange(128)[:, None]
    ii = np.arange(512)[None, :]
    masks = np.concatenate(
        [(ii >= jj + r * 128).astype(np.float32) for r in range(4)], axis=1
    ).astype(bf)
    ones_col = np.ones((128, 1), dtype=bf)
    ones_row = np.ones((1, 128), dtype=bf)

    xTb = [np.ascontiguousarray(x[b].T).astype(bf) for b in range(B)]
    in_maps = []
    for c in range(8):
        b, g = divmod(c, 4)
        sl = slice(g * HG, (g + 1) * HG)
        bqk = np.concatenate(
            [np.asarray(bq)[sl].reshape(NH, 128).T,
             np.asarray(bk)[sl].reshape(NH, 128).T], axis=1
        ).astype(np.float32)
        bv_rep = np.broadcast_to(
            np.asarray(bv)[sl].astype(np.float32), (128, HG)).copy()
        in_maps.append({
            "xT": xTb[b],
            "wq": np.ascontiguousarray(np.asarray(Wq)[:, sl]).astype(bf),
            "wk": np.ascontiguousarray(np.asarray(Wk)[:, sl]).astype(bf),
            "wv": np.ascontiguousarray(np.asarray(Wv)[:, sl]).astype(bf),
            "wp": np.ascontiguousarray(np.asarray(Wp)[sl, :]).astype(bf),
            "bqk": bqk,
            "bv": bv_rep,
            "masks": masks,
            "ones_col": ones_col,
            "ones_row": ones_row,
        })
    return in_maps


def kernel(x, Wq, bq, Wk, bk, Wv, bv, Wp, bp):
    global LAST_EXEC_NS
    # NTFF tracing needs antenv.axon_hooks, absent in this container; a set
    # BASS_TRACE would crash run_bass_kernel_spmd otherwise.
    os.environ["BASS_NEVER_TRACE"] = "1"
    nc = _get_nc()
    in_maps = _prep_in_maps(x, Wq, bq, Wk, bk, Wv, bv, Wp, bp)
    res = run_bass_kernel_spmd(nc, in_maps, core_ids=list(range(8)))
    LAST_EXEC_NS = res.exec_time_ns
    out = np.empty((B, S, D), dtype=np.float32)
    for b in range(B):
        acc = res.results[4 * b]["out"].astype(np.float32)
        for g in range(1, 4):
            acc = acc + res.results[4 * b + g]["out"].astype(np.float32)
        out[b] = acc
    out += np.asarray(bp, dtype=np.float32)[None, None, :]
    return out


def _make_runner(nc, in_maps):
    """Replicate bass2jax.run_bass_via_pjrt's shard_map jit, returning a
    zero-arg callable over device-resident inputs (for repeat timing)."""
    import jax
    from jax.sharding import Mesh, PartitionSpec, NamedSharding
    from jax.experimental.shard_map import shard_map
    from concourse import bass2jax, mybir as _mybir
    from concourse.bass2jax import _bass_exec_p, install_neuronx_cc_hook

    install_neuronx_cc_hook()
    n_cores = len(in_maps)
    partition_name = (nc.partition_id_tensor.name
                      if nc.partition_id_tensor else None)
    in_names, out_names, out_avals, zero_outs = [], [], [], []
    for alloc in nc.m.functions[0].allocations:
        if not isinstance(alloc, _mybir.MemoryLocationSet):
            continue
        name = alloc.memorylocations[0].name
        if alloc.kind == "ExternalInput":
            if name != partition_name:
                in_names.append(name)
        elif alloc.kind == "ExternalOutput":
            out_names.append(name)
            shape = tuple(alloc.tensor_shape)
            dtype = _mybir.dt.np(alloc.dtype)
            out_avals.append(jax.core.ShapedArray(shape, dtype))
            zero_outs.append(np.zeros(shape, dtype))
    n_params = len(in_names)
    n_outs = len(out_avals)
    in_names = in_names + out_names
    if partition_name is not None:
        in_names.append(partition_name)

    def _body(*args):
        operands = list(args)
        if partition_name is not None:
            operands.append(bass2jax.partition_id_tensor())
        outs = _bass_exec_p.bind(
            *operands, out_avals=tuple(out_avals), in_names=tuple(in_names),
            out_names=tuple(out_names), lowering_input_output_aliases=(),
            sim_require_finite=True, sim_require_nnan=True, nc=nc)
        return tuple(outs)

    devices = jax.devices()[:n_cores]
    mesh = Mesh(np.asarray(devices), ("core",))
    in_specs = (PartitionSpec("core"),) * (n_params + n_outs)
    out_specs = (PartitionSpec("core"),) * len(out_names)
    fn = jax.jit(
        shard_map(_body, mesh=mesh, in_specs=in_specs, out_specs=out_specs,
                  check_rep=False),
        keep_unused=True)
    sh = NamedSharding(mesh, PartitionSpec("core"))
    concat_in = [
        jax.device_put(
            np.concatenate([np.asarray(in_maps[c][in_names[i]])
                            for c in range(n_cores)], axis=0), sh)
        for i in range(n_params)
    ]
    concat_zeros = [
        jax.device_put(np.zeros((n_cores * z.shape[0], *z.shape[1:]), z.dtype), sh)
        for z in zero_outs
    ]
    args = concat_in + concat_zeros

    def run():
        return fn(*args)

    return run


def _time_runner(run, iters):
    import time
    import jax
    jax.block_until_ready(run())  # compile + warm
    times = []
    for _ in range(iters):
        t0 = time.perf_counter()
        jax.block_until_ready(run())
        times.append(time.perf_counter() - t0)
    times.sort()
    return times


def benchmark(inputs, iters=12, loop_n=32):
    """Estimate per-execution HW time by amplifying the kernel body with an
    on-device For_i loop: t = (wall(loop_n) - wall(1)) / (loop_n - 1).
    Tunnel RPC overhead (~100 ms) cancels in the difference."""
    in_maps = _prep_in_maps(**inputs)
    run1 = _make_runner(_get_nc(1), in_maps)
    runN = _make_runner(_get_nc(loop_n), in_maps)
    t1 = _time_runner(run1, iters)
    tN = _time_runner(runN, iters)
    med1 = t1[len(t1) // 2]
    medN = tN[len(tN) // 2]
    est = (medN - med1) / (loop_n - 1)
    print(f"benchmark: wall(1) med {med1*1e3:.1f} ms, wall({loop_n}) med "
          f"{medN*1e3:.1f} ms -> est {est*1e6:.0f} us/exec")
    return est * 1e9

